# revision 69
# baseline (speedup 1.0000x reference)
"""LIF layer (leaky integrate-and-fire scan over time) on 8 Trainium2 cores.

Recurrence per (b, f) row over t = 0..L-1:
    v_pre[t] = alpha[f] * v[t-1] + (1 - alpha[f]) * I[b, f, t]
    z[t]     = BETA * (v_pre[t] - THR)
    s[t]     = (v_pre[t] >= THR)
    v[t]     = v_pre[t] * (v_pre[t] < THR)          # reset on spike

Outputs: (v_pre, z, s) each [B, F, L] float32 -- note all three are pure
elementwise functions of v_pre, so the device only emits v_pre and the
host derives v/z/s.

v4 (main path) design:
- Rescaled state vt = v/(1-alpha): vt_pre = alpha*vt + I feeds on raw
  input (no J precompute); spike iff vt_pre >= THR/(1-alpha) (a
  per-partition threshold), so both chain ops are single fused
  scalar_tensor_tensor DVE instructions with per-partition scalars.
- Time-sharded: per f-half (128 partitions) the 2048 steps split into
  segments, each scanned independently after w=16 discarded warmup steps
  (the contraction alpha<1 re-converges the state from v=0; validated
  against the reference at rel err ~1.6e-3, tolerance 2e-2).
- Per core, two engine lanes run concurrently:
  * DVE: 2 interleaved chains (hides the per-op tick-semaphore RTT),
    each packing 2 segments side-by-side in the free dim ([128, 2*64]
    per step, l_da/l_db = 112/96 steps).
  * GpSimd: 2 interleaved chains (l_p=48) using only Pool-legal ops,
    3 per step: m=(vp<thr)*alpha [tensor_scalar]; m=m*vp; vp'=m+I
    [tensor_tensor], writing into a persistent SBUF history that both
    serves as the recurrence state and is DMA'd out (no WAR hazards).
- Outputs: DVE lane emits y = vt_pre - thr_t in bf16 via one ACT
  Identity op per chunk (bias = -thr_t per partition); the bf16 sign
  bit (incl. signed zero) is the exact spike decision, so s is
  bit-correct while v/z carry only ~0.4% bf16 rounding. Pool lane emits
  f32 directly. This cuts DMA traffic below the serial-DMA roofline.
- Ring discipline: SP ring carries only always-ready input DMAs
  (first chunks split into pieces so compute starts early; tiny const
  loads ride the ACT ring so they are not queued behind megabyte
  inputs); outputs ride the ACT ring; pool outputs flush on SP two
  chunks late / at the tail. Chunk streams of the two lanes are emitted
  merged in expected-consumption order; deep tile pools keep every
  ring head free of unsatisfied waits.

TimelineSim (the graded cost model): 103411 ns vs 320986 ns baseline.
"""

import sys

sys.path.insert(0, "/opt/trn_rl_repo")

import numpy as np

DT = 1.0
BETA = 15.0
THR = 0.25

B, F, L = 64, 256, 2048
SB, SF = 4, 2  # B-split x F-split = 8 cores
BL, FL = B // SB, F // SF  # 16, 128
TC = 256  # time-chunk length
N_CORES = SB * SF

_BUILD_CACHE: dict = {}
LAST_RESULTS = None  # BassKernelResults of the most recent kernel() call


def _build(bl: int, fl: int, ll: int, tc: int):
    """Build the per-core Bass program (same NEFF for all cores)."""
    import concourse.bacc as bacc
    import concourse.mybir as mybir
    from concourse import tile

    f32 = mybir.dt.float32
    Alu = mybir.AluOpType
    Act = mybir.ActivationFunctionType

    nchunk = ll // tc
    assert ll % tc == 0

    nc = bacc.Bacc(None, target_bir_lowering=False)
    i_d = nc.dram_tensor("i_loc", [fl, bl, ll], f32, kind="ExternalInput")
    al_d = nc.dram_tensor("alpha", [fl, 1], f32, kind="ExternalInput")
    om_d = nc.dram_tensor("omalpha", [fl, 1], f32, kind="ExternalInput")
    v_d = nc.dram_tensor("v_out", [fl, bl, ll], f32, kind="ExternalOutput")
    z_d = nc.dram_tensor("z_out", [fl, bl, ll], f32, kind="ExternalOutput")
    s_d = nc.dram_tensor("s_out", [fl, bl, ll], f32, kind="ExternalOutput")

    with tile.TileContext(nc) as tc_:
        with (
            tc_.tile_pool(name="const", bufs=1) as constp,
            tc_.tile_pool(name="io", bufs=2) as iop,
        ):
            al_t = constp.tile([fl, 1], f32, tag="al")
            om_t = constp.tile([fl, 1], f32, tag="om")
            nc.sync.dma_start(al_t[:], al_d[:])
            nc.sync.dma_start(om_t[:], om_d[:])

            vst = constp.tile([fl, bl], f32, tag="vst")
            nc.gpsimd.memset(vst[:], 0.0)

            for k in range(nchunk):
                tsl = slice(k * tc, (k + 1) * tc)

                it = iop.tile([fl, bl, tc], f32, tag="i")
                nc.sync.dma_start(it[:], i_d[:, :, tsl])

                # J = (1 - alpha) * I  (single-rounded FMA on ScalarE; same
                # result as the reference's f32 multiply)
                jt = iop.tile([fl, bl, tc], f32, tag="j")
                nc.scalar.activation(jt[:], it[:], Act.Copy, bias=0.0, scale=om_t[:, 0:1])

                vp = iop.tile([fl, bl, tc], f32, tag="vp")
                for t in range(tc):
                    # v_pre = (v * alpha) + J_t
                    nc.vector.scalar_tensor_tensor(
                        vp[:, :, t], vst[:], al_t[:, 0:1], jt[:, :, t],
                        op0=Alu.mult, op1=Alu.add,
                    )
                    # v = (v_pre < thr) * v_pre
                    nc.vector.scalar_tensor_tensor(
                        vst[:], vp[:, :, t], THR, vp[:, :, t],
                        op0=Alu.is_lt, op1=Alu.mult,
                    )

                # z = (v_pre - thr) * BETA   (reference rounding order)
                zt = iop.tile([fl, bl, tc], f32, tag="z")
                nc.gpsimd.tensor_scalar(zt[:], vp[:], THR, BETA, Alu.subtract, Alu.mult)
                # s = (v_pre >= thr)
                st = iop.tile([fl, bl, tc], f32, tag="s")
                nc.gpsimd.tensor_scalar(st[:], vp[:], THR, None, Alu.is_ge)

                nc.sync.dma_start(v_d[:, :, tsl], vp[:])
                nc.sync.dma_start(z_d[:, :, tsl], zt[:])
                nc.sync.dma_start(s_d[:, :, tsl], st[:])

    nc.compile()
    return nc


def _get_nc():
    key = (BL, FL, L, TC)
    if key not in _BUILD_CACHE:
        _BUILD_CACHE[key] = _build(*key)
    return _BUILD_CACHE[key]


def _build_v2(bl: int, fl: int, tseg: int, w: int, tc: int):
    """Time-sharded build: 8 cores = 2 f-halves x 4 time segments.

    Each core scans w warmup steps (converging the decaying state from
    v=0; seg 0 gets zero-padded input so the NEFF is uniform) and then
    tseg output steps. Serial chain: 2 fused STT DVE ops per step at
    free-dim = bl.

    All DRAM I/O is slab-major — [fl, n_slabs, bl, tc] — so every DMA
    moves one whole [fl, bl*tc] tile as 128 contiguous per-partition
    slabs (16KB descriptors), letting short chunks stream without the
    sub-512B descriptor penalty. The host packs/unpacks the layout.
    """
    import concourse.bacc as bacc
    import concourse.mybir as mybir
    from concourse import tile

    f32 = mybir.dt.float32
    Alu = mybir.AluOpType
    Act = mybir.ActivationFunctionType

    tt = w + tseg
    assert tt % tc == 0 and w % tc == 0
    nw, ns = w // tc, tseg // tc

    nc = bacc.Bacc(None, target_bir_lowering=False)
    i_d = nc.dram_tensor("i_loc", [fl, nw + ns, bl, tc], f32, kind="ExternalInput")
    al_d = nc.dram_tensor("alpha", [fl, 1], f32, kind="ExternalInput")
    om_d = nc.dram_tensor("omalpha", [fl, 1], f32, kind="ExternalInput")
    v_d = nc.dram_tensor("v_out", [fl, ns, bl, tc], f32, kind="ExternalOutput")
    z_d = nc.dram_tensor("z_out", [fl, ns, bl, tc], f32, kind="ExternalOutput")
    s_d = nc.dram_tensor("s_out", [fl, ns, bl, tc], f32, kind="ExternalOutput")

    with tile.TileContext(nc) as tc_:
        with (
            tc_.tile_pool(name="const", bufs=1) as constp,
            tc_.tile_pool(name="io", bufs=3) as iop,
            tc_.tile_pool(name="zs", bufs=2) as zsp,
        ):
            al_t = constp.tile([fl, 1], f32, tag="al")
            om_t = constp.tile([fl, 1], f32, tag="om")
            nc.sync.dma_start(al_t[:], al_d[:])
            nc.sync.dma_start(om_t[:], om_d[:])

            vst = constp.tile([fl, bl], f32, tag="vst")
            nc.gpsimd.memset(vst[:], 0.0)
            vp_w = constp.tile([fl, bl], f32, tag="vpw")  # warmup v_pre slot

            for k in range(nw + ns):
                is_out = k >= nw
                it = iop.tile([fl, bl, tc], f32, tag="i")
                nc.sync.dma_start(it[:], i_d[:, k])
                # J = (1 - alpha) * I, in place over the input tile
                nc.scalar.activation(it[:], it[:], Act.Copy, bias=0.0, scale=om_t[:, 0:1])

                if not is_out:  # warmup chunk: no outputs
                    for t in range(tc):
                        nc.vector.scalar_tensor_tensor(
                            vp_w[:], vst[:], al_t[:, 0:1], it[:, :, t],
                            op0=Alu.mult, op1=Alu.add,
                        )
                        nc.vector.scalar_tensor_tensor(
                            vst[:], vp_w[:], THR, vp_w[:],
                            op0=Alu.is_lt, op1=Alu.mult,
                        )
                    continue

                last = k == nw + ns - 1
                o = k - nw
                vp = iop.tile([fl, bl, tc], f32, tag="vp")
                for t in range(tc):
                    nc.vector.scalar_tensor_tensor(
                        vp[:, :, t], vst[:], al_t[:, 0:1], it[:, :, t],
                        op0=Alu.mult, op1=Alu.add,
                    )
                    nc.vector.scalar_tensor_tensor(
                        vst[:], vp[:, :, t], THR, vp[:, :, t],
                        op0=Alu.is_lt, op1=Alu.mult,
                    )

                # z = (vp - thr) * beta, s = (vp >= thr): bulk on GpSimd
                # mid-stream (hidden behind the DVE chain); on DVE for the
                # final chunk so the tail isn't gated on slow GpSimd passes.
                eng = nc.vector if last else nc.gpsimd
                zt = zsp.tile([fl, bl, tc], f32, tag="z")
                eng.tensor_scalar(zt[:], vp[:], THR, BETA, Alu.subtract, Alu.mult)
                st = zsp.tile([fl, bl, tc], f32, tag="s")
                eng.tensor_scalar(st[:], vp[:], THR, None, Alu.is_ge)

                # Outputs ride the ACT HWDGE ring so they never queue ahead
                # of the next input chunk on the SP ring (FIFO per ring).
                nc.scalar.dma_start(v_d[:, o], vp[:])
                nc.scalar.dma_start(z_d[:, o], zt[:])
                nc.scalar.dma_start(s_d[:, o], st[:])

    nc.compile()
    return nc


def _pick_warmup(alpha: np.ndarray) -> int:
    """Steps for the state to converge below fp32 resolution from v=0,
    with ~2x margin for spike-flip self-healing. Multiple of 128."""
    amax = float(alpha.max())
    amax = min(max(amax, 1e-6), 0.999999)
    wraw = 2.2 * np.log(4e-10) / np.log(amax)
    w = int(np.ceil(max(wraw, 1.0) / 128.0)) * 128
    return max(w, 128)


def _alpha_host(raw_tau: np.ndarray) -> tuple[np.ndarray, np.ndarray]:
    """alpha = exp(-DT / (softplus(raw_tau) + 1e-4)) with the same jax ops /
    device as the reference, so spike threshold comparisons match bitwise."""
    import jax
    import jax.numpy as jnp

    with jax.default_device(jax.devices("cpu")[0]):
        tau = jax.nn.softplus(jnp.asarray(np.asarray(raw_tau))) + 1e-4
        alpha = np.asarray(jnp.exp(-DT / tau), dtype=np.float32)
    one_minus = (np.float32(1.0) - alpha).astype(np.float32)
    return alpha, one_minus


def _build_v3(bl: int, fl: int, g: int, w: int, tseg: int, tc: int):
    """v3: rescaled recurrence, single output, C=2 chains x G packed units.

    Rescale: vt = v / (1-alpha)  =>  vt_pre = alpha*vt + I ; spike iff
    vt_pre >= thr/(1-alpha) =: thr_t (per-partition). Only vt_pre is
    written out; the host derives v = (1-alpha)*vt_pre, z, s.

    Each core runs 2 interleaved chains on DVE (hides the tick-sem RTT);
    each chain packs g independent (f-half, segment) units side-by-side in
    the free dim ([fl, g*bl] per step) so per-instruction overhead
    amortizes. Compute is in-place over the input tile: vp[t] overwrites
    I[t], and the tile is then DMA'd out as the output chunk.
    """
    import concourse.bacc as bacc
    import concourse.mybir as mybir
    from concourse import tile

    f32 = mybir.dt.float32
    Alu = mybir.AluOpType

    tt = w + tseg
    assert tt % tc == 0 and w % tc == 0
    nw, ns = w // tc, tseg // tc
    fr = g * bl  # free width per chain step

    nc = bacc.Bacc(None, target_bir_lowering=False)
    i_d = nc.dram_tensor("i_loc", [fl, 2, nw + ns, tc, fr], f32, kind="ExternalInput")
    al_d = nc.dram_tensor("alpha", [fl, 1], f32, kind="ExternalInput")
    th_d = nc.dram_tensor("thr_t", [fl, 1], f32, kind="ExternalInput")
    o_d = nc.dram_tensor("vp_out", [fl, 2, ns, tc, fr], f32, kind="ExternalOutput")

    with tile.TileContext(nc) as tc_:
        with (
            tc_.tile_pool(name="const", bufs=1) as constp,
            tc_.tile_pool(name="io", bufs=3) as iop,
        ):
            al_t = constp.tile([fl, 1], f32, tag="al")
            th_t = constp.tile([fl, 1], f32, tag="th")
            nth_t = constp.tile([fl, 1], f32, tag="nth")
            # al/th ride the Pool SWDGE ring: independent of the SP/ACT
            # rings so the input quarters are not displaced. nth is only
            # needed by the ACT conversions (~2 chunks in) -> ACT ring.
            nc.gpsimd.dma_start(al_t[:], al_d[:])
            nc.gpsimd.dma_start(th_t[:], th_d[:])
            nc.scalar.dma_start(nth_t[:], nth_d[:])

            vst = [
                constp.tile([fl, fr], f32, tag=f"vst{c}", name=f"vst{c}")
                for c in range(2)
            ]
            for c in range(2):
                nc.gpsimd.memset(vst[c][:], 0.0)

            for k in range(nw + ns):
                is_out = k >= nw
                its = []
                for c in range(2):
                    it = iop.tile([fl, tc, fr], f32, tag=f"i{c}", name=f"i{c}_{k}")
                    nc.sync.dma_start(it[:], i_d[:, c, k])
                    its.append(it)
                for t in range(tc):
                    for c in range(2):
                        # vp = alpha*v + I  (in place over the input slot)
                        nc.vector.scalar_tensor_tensor(
                            its[c][:, t], vst[c][:], al_t[:, 0:1], its[c][:, t],
                            op0=Alu.mult, op1=Alu.add,
                        )
                        # v = (vp < thr_t) * vp
                        nc.vector.scalar_tensor_tensor(
                            vst[c][:], its[c][:, t], th_t[:, 0:1], its[c][:, t],
                            op0=Alu.is_lt, op1=Alu.mult,
                        )
                if is_out:
                    for c in range(2):
                        nc.scalar.dma_start(o_d[:, c, k - nw], its[c][:])

    nc.compile()
    return nc


def _run_v3(I, alpha, thr_t, one_minus, w, _trace):
    global LAST_RESULTS, _CURRENT_NC
    from concourse.bass_utils import run_bass_kernel_spmd

    g = 2
    nseg = 16  # 2 chains x g units on each of 8 cores, x 2 f-halves
    tseg = L // nseg  # 128
    bl3, fl3 = B, 128
    tc = 16
    assert w % tc == 0

    key = ("v3", bl3, fl3, g, w, tseg, tc)
    if key not in _BUILD_CACHE:
        _BUILD_CACHE[key] = _build_v3(bl3, fl3, g, w, tseg, tc)
    nc = _BUILD_CACHE[key]
    _CURRENT_NC = nc

    nck = (w + tseg) // tc
    fr = g * bl3
    # unit u (0..31): f-half = u % 2, segment = u // 2. Core/chain/slot:
    # core c handles units with u//2 in [4c//2..), chain layout below.
    in_maps = []
    for c in range(N_CORES):
        fg = c % 2
        fsl = slice(fg * fl3, (fg + 1) * fl3)
        q = c // 2  # quarter 0..3: segments 4q..4q+3
        i_loc = np.zeros((fl3, 2, nck, tc, g, bl3), np.float32)
        for ch in range(2):
            for u in range(g):
                seg = 4 * q + 2 * ch + u
                t0 = seg * tseg
                lo = max(0, t0 - w)
                pad = np.zeros((fl3, bl3, w + tseg), np.float32)
                pad[:, :, w - (t0 - lo):] = I[:, fsl, lo : t0 + tseg].transpose(1, 0, 2)
                # [fl, bl, T] -> [fl, nck, tc, bl] into slot u
                v = pad.reshape(fl3, bl3, nck, tc).transpose(0, 2, 3, 1)
                i_loc[:, ch, :, :, u, :] = v
        i_loc = i_loc.reshape(fl3, 2, nck, tc, fr)
        in_maps.append(
            {
                "i_loc": i_loc,
                "alpha": np.ascontiguousarray(alpha[fsl].reshape(fl3, 1)),
                "thr_t": np.ascontiguousarray(thr_t[fsl].reshape(fl3, 1)),
            }
        )

    res = run_bass_kernel_spmd(nc, in_maps, core_ids=list(range(N_CORES)), trace=_trace)
    LAST_RESULTS = res

    vp = np.empty((B, F, L), np.float32)
    for c in range(N_CORES):
        fg = c % 2
        fsl = slice(fg * fl3, (fg + 1) * fl3)
        q = c // 2
        r = res.results[c]["vp_out"]  # [fl, 2, ns, tc, fr]
        ns_ = (w + tseg) // tc - w // tc
        rr = r.reshape(fl3, 2, ns_, tc, g, bl3)
        for ch in range(2):
            for u in range(g):
                seg = 4 * q + 2 * ch + u
                t0 = seg * tseg
                a = rr[:, ch, :, :, u, :].transpose(3, 0, 1, 2).reshape(bl3, fl3, tseg)
                vp[:, fsl, t0 : t0 + tseg] = a

    # host-side: derive the three outputs from vt_pre
    om = one_minus.reshape(1, F, 1)
    v = (vp * om).astype(np.float32)
    z = ((v - np.float32(THR)) * np.float32(BETA)).astype(np.float32)
    s = (vp >= thr_t.reshape(1, F, 1)).astype(np.float32)
    return v, z, s


USE_V2 = True
_CURRENT_NC = None


def _get_current_nc():
    return _CURRENT_NC


def _run_v1(I, alpha, one_minus, _trace):
    global LAST_RESULTS, _CURRENT_NC
    from concourse.bass_utils import run_bass_kernel_spmd

    nc = _get_nc()
    _CURRENT_NC = nc

    in_maps = []
    for c in range(N_CORES):
        fg, bg = c % SF, c // SF
        fsl = slice(fg * FL, (fg + 1) * FL)
        bsl = slice(bg * BL, (bg + 1) * BL)
        i_loc = np.ascontiguousarray(I[bsl, fsl, :].transpose(1, 0, 2))  # [FL, BL, L]
        in_maps.append(
            {
                "i_loc": i_loc,
                "alpha": np.ascontiguousarray(alpha[fsl].reshape(FL, 1)),
                "omalpha": np.ascontiguousarray(one_minus[fsl].reshape(FL, 1)),
            }
        )

    res = run_bass_kernel_spmd(nc, in_maps, core_ids=list(range(N_CORES)), trace=_trace)
    LAST_RESULTS = res

    v = np.empty((B, F, L), np.float32)
    z = np.empty((B, F, L), np.float32)
    s = np.empty((B, F, L), np.float32)
    for c in range(N_CORES):
        fg, bg = c % SF, c // SF
        fsl = slice(fg * FL, (fg + 1) * FL)
        bsl = slice(bg * BL, (bg + 1) * BL)
        r = res.results[c]
        v[bsl, fsl, :] = r["v_out"].transpose(1, 0, 2)
        z[bsl, fsl, :] = r["z_out"].transpose(1, 0, 2)
        s[bsl, fsl, :] = r["s_out"].transpose(1, 0, 2)
    return v, z, s


def _run_v2(I, alpha, one_minus, w, _trace):
    global LAST_RESULTS, _CURRENT_NC
    from concourse.bass_utils import run_bass_kernel_spmd

    nseg = 4
    tseg = L // nseg  # 512
    bl2, fl2, tc = B, 128, 64  # all of B, half of F per core

    key = ("v2", bl2, fl2, tseg, w, tc)
    if key not in _BUILD_CACHE:
        _BUILD_CACHE[key] = _build_v2(bl2, fl2, tseg, w, tc)
    nc = _BUILD_CACHE[key]
    _CURRENT_NC = nc

    nck = (w + tseg) // tc
    in_maps = []
    for c in range(N_CORES):
        fg, seg = c % 2, c // 2
        fsl = slice(fg * fl2, (fg + 1) * fl2)
        t0 = seg * tseg
        i_pad = np.zeros((fl2, bl2, w + tseg), np.float32)
        lo = max(0, t0 - w)
        i_pad[:, :, w - (t0 - lo):] = I[:, fsl, lo : t0 + tseg].transpose(1, 0, 2)
        i_sm = i_pad.reshape(fl2, bl2, nck, tc).transpose(0, 2, 1, 3)
        in_maps.append(
            {
                "i_loc": np.ascontiguousarray(i_sm),
                "alpha": np.ascontiguousarray(alpha[fsl].reshape(fl2, 1)),
                "omalpha": np.ascontiguousarray(one_minus[fsl].reshape(fl2, 1)),
            }
        )

    res = run_bass_kernel_spmd(nc, in_maps, core_ids=list(range(N_CORES)), trace=_trace)
    LAST_RESULTS = res

    v = np.empty((B, F, L), np.float32)
    z = np.empty((B, F, L), np.float32)
    s = np.empty((B, F, L), np.float32)
    for c in range(N_CORES):
        fg, seg = c % 2, c // 2
        fsl = slice(fg * fl2, (fg + 1) * fl2)
        t0 = seg * tseg
        r = res.results[c]
        for name, dst in (("v_out", v), ("z_out", z), ("s_out", s)):
            a = r[name].transpose(2, 0, 1, 3).reshape(bl2, fl2, tseg)
            dst[:, fsl, t0 : t0 + tseg] = a
    return v, z, s


def _build_v4(bl: int, fl: int, w: int, l_da: int, l_db: int, l_p: int, tc: int, tc_p: int = 0):
    """v4 = v3 + a GpSimd (Pool) chain lane running in parallel with DVE.

    DVE lane: 2 interleaved chains x 2 packed units (free=2*bl), 2 STT
    ops/step, segments of length l_d.
    Pool lane: 2 interleaved chains x 1 unit (free=bl), segments of length
    l_p, 3 ops/step using only Pool-legal instructions:
        m  = (vp < thr_t) * alpha      # tensor_scalar, per-partition scalars
        m  = m * vp                    # tensor_tensor   (= alpha * v)
        vp' = m + I_t                  # tensor_tensor, in place over I_t
    The Pool state is vp (pre-reset), stored in the input tile slots, so
    output chunks are DMA'd straight out. Coverage per core:
    4*l_d + 2*l_p output steps.
    """
    import concourse.bacc as bacc
    import concourse.mybir as mybir
    from concourse import tile

    f32 = mybir.dt.float32
    bf16 = mybir.dt.bfloat16
    i8 = mybir.dt.int8
    Alu = mybir.AluOpType
    Act = mybir.ActivationFunctionType

    tc_p = tc_p or tc
    assert w % tc == 0 and (w + l_da) % tc == 0 and (w + l_db) % tc == 0
    assert w % tc_p == 0 and (w + l_p) % tc_p == 0
    nw = w // tc
    nda = (w + l_da) // tc - nw
    ndb = (w + l_db) // tc - nw
    nwp, npo = w // tc_p, (w + l_p) // tc_p - w // tc_p
    frd = 2 * bl

    nc = bacc.Bacc(None, target_bir_lowering=False)
    ida_d = nc.dram_tensor("i_dvea", [fl, nw + nda, tc, frd], f32, kind="ExternalInput")
    idb_d = nc.dram_tensor("i_dveb", [fl, nw + ndb, tc, frd], f32, kind="ExternalInput")
    ip_d = nc.dram_tensor("i_pool", [fl, 2, nwp + npo, tc_p, bl], f32, kind="ExternalInput")
    sc_d = nc.dram_tensor("scal2", [fl, 2], f32, kind="ExternalInput")
    nth_d = nc.dram_tensor("nthr_t", [fl, 1], f32, kind="ExternalInput")
    oda_d = nc.dram_tensor("ya_out", [fl, nda, tc, frd], bf16, kind="ExternalOutput")
    odb_d = nc.dram_tensor("yb_out", [fl, ndb, tc, frd], bf16, kind="ExternalOutput")
    op_d = nc.dram_tensor("vpp_out", [fl, 2, npo, tc_p, bl], f32, kind="ExternalOutput")
    id_ds = [ida_d, idb_d]
    od_ds = [oda_d, odb_d]
    nds = [nda, ndb]

    with tile.TileContext(nc) as tc_:
        with (
            tc_.tile_pool(name="const", bufs=1) as constp,
            tc_.tile_pool(name="iod", bufs=globals().get("_IOD_BUFS", 8)) as iod,
            tc_.tile_pool(name="iop2", bufs=globals().get("_IOP_BUFS", 4)) as iop2,
            tc_.tile_pool(name="ocast", bufs=2) as ocp,
        ):
            sc_t = constp.tile([fl, 2], f32, tag="sc")
            nth_t = constp.tile([fl, 1], f32, tag="nth")
            # alpha+thr packed in one tiny transfer on the Pool SWDGE ring:
            # one desc-gen, independent of the SP/ACT rings, lands ~2us
            # before the first input piece. nth is only needed by the ACT
            # conversions (~2 chunks in) -> ACT ring.
            nc.gpsimd.dma_start(sc_t[:], sc_d[:])
            nc.scalar.dma_start(nth_t[:], nth_d[:])
            al_ap = sc_t[:, 0:1]
            th_ap = sc_t[:, 1:2]

            vst = [
                constp.tile([fl, frd], f32, tag=f"vst{c}", name=f"vst{c}")
                for c in range(2)
            ]
            pm = [
                constp.tile([fl, bl], f32, tag=f"pm{c}", name=f"pm{c}")
                for c in range(2)
            ]
            # Pool lane: persistent output buffer (doubles as the state
            # history) + a 2-slot scratch ring for warmup steps.
            pout = [
                constp.tile([fl, npo * tc_p, bl], f32, tag=f"pout{c}", name=f"pout{c}")
                for c in range(2)
            ]
            pscr = [
                constp.tile([fl, 2, bl], f32, tag=f"pscr{c}", name=f"pscr{c}")
                for c in range(2)
            ]
            # no memsets needed: step 0 of every chain writes its state
            # before anything reads it (v[-1]=0 is folded into step 0)

            # Merge the two lanes' chunk streams in expected-consumption
            # order so neither lane's input DMAs queue behind the other's
            # entire stream on the shared SP ring (FIFO per ring).
            t_step_d = 2 * 2 * 213.0   # 2 chains x 2 STT ops per step, ns
            t_step_p = 2 * 3 * 228.0   # 2 chains x 3 pool ops per step
            # in-DMAs are separate events emitted PRE chunks ahead of their
            # compute so ring order leads consumption by a fixed margin.
            PRE = globals().get("_PRE_OVERRIDE", 1)
            nkd = nw + max(nda, ndb)
            nkp = nwp + npo
            prio = {"din": 0, "pin": 1, "d": 2, "p": 3}
            sched = [("din", k, max(k - PRE, 0) * tc * t_step_d - 1) for k in range(nkd)]
            sched += [("d", k, k * tc * t_step_d) for k in range(nkd)]
            sched += [("pin", k, max(k - PRE, 0) * tc_p * t_step_p - 1) for k in range(nkp)]
            sched += [("p", k, max(k - 1, 0) * tc_p * t_step_p) for k in range(nkp)]
            sched.sort(key=lambda e: (e[2], e[1], prio[e[0]]))

            dtiles: dict = {}
            ptiles: dict = {}
            for lane, k, _ in sched:
                is_out = k >= nw
                if lane == "din":
                    cs = [c for c in range(2) if k < nw + nds[c]]
                    dtiles[k] = {}
                    for c in cs:
                        dtiles[k][c] = iod.tile(
                            [fl, tc, frd], f32, tag=f"i{c}", name=f"i{c}_{k}"
                        )
                    if k == 0:
                        q4 = tc // 4
                        for qi in range(4):
                            for c in cs:
                                nc.sync.dma_start(
                                    dtiles[k][c][:, qi * q4 : (qi + 1) * q4],
                                    id_ds[c][:, k, qi * q4 : (qi + 1) * q4],
                                )
                    elif k == 1:
                        h = tc // 2
                        for hi in range(2):
                            for c in cs:
                                nc.sync.dma_start(
                                    dtiles[k][c][:, hi * h : (hi + 1) * h],
                                    id_ds[c][:, k, hi * h : (hi + 1) * h],
                                )
                    else:
                        for c in cs:
                            nc.sync.dma_start(dtiles[k][c][:], id_ds[c][:, k])

                elif lane == "pin":
                    pts = []
                    for c in range(2):
                        pt = iop2.tile([fl, tc_p, bl], f32, tag=f"p{c}", name=f"p{c}_{k}")
                        nc.sync.dma_start(pt[:], ip_d[:, c, k])
                        pts.append(pt)
                    ptiles[k] = pts
                elif lane == "d":
                    if k == nw + max(nda, ndb) - 2:
                        # pool tail flush: pool compute is done (or nearly);
                        # nothing queues behind these on SP anymore
                        for j in range(max(npo - 2, 0), npo):
                            for c in range(2):
                                sl = slice(j * tc_p, (j + 1) * tc_p)
                                nc.sync.dma_start(op_d[:, c, j], pout[c][:, sl])
                    cs = [c for c in range(2) if k < nw + nds[c]]
                    its = dtiles.pop(k)
                    for t in range(tc):
                        if not (k == 0 and t == 0):
                            # vp = alpha*v + I; at the very first step v=0
                            # so vp == I and the tile slot is already right
                            for c in cs:
                                nc.vector.scalar_tensor_tensor(
                                    its[c][:, t], vst[c][:], al_ap, its[c][:, t],
                                    op0=Alu.mult, op1=Alu.add,
                                )
                        for c in cs:
                            nc.vector.scalar_tensor_tensor(
                                vst[c][:], its[c][:, t], th_ap, its[c][:, t],
                                op0=Alu.is_lt, op1=Alu.mult,
                            )
                        for c in cs:
                            # last chunk: shift-cast + out in 3/4 + 1/4 so
                            # the end-of-program drain is one quarter
                            if k == nw + nds[c] - 1 and t == 3 * tc // 4 - 1 and is_out:
                                q3 = 3 * tc // 4
                                ob = ocp.tile([fl, tc, frd], bf16, tag=f"ob{c}", name=f"ob{c}_{k}")
                                its[c + 2] = ob
                                nc.scalar.activation(ob[:, :q3], its[c][:, :q3], Act.Identity, bias=nth_t[:, 0:1], scale=1.0)
                                nc.scalar.dma_start(od_ds[c][:, k - nw, :q3], ob[:, :q3])
                    if is_out:
                        for c in cs:
                            if k == nw + nds[c] - 1:
                                q3 = 3 * tc // 4
                                ob = its[c + 2]
                                if k == nw + max(nda, ndb) - 1:
                                    # global last piece: convert on DVE right
                                    # after the chain (no cross-engine hop)
                                    # and ship on the idle SP ring (shorter
                                    # DGE delay); nothing queues behind it
                                    nc.vector.tensor_scalar(
                                        ob[:, q3:], its[c][:, q3:],
                                        nth_t[:, 0:1], None, Alu.add,
                                    )
                                    nc.sync.dma_start(od_ds[c][:, k - nw, q3:], ob[:, q3:])
                                else:
                                    nc.scalar.activation(ob[:, q3:], its[c][:, q3:], Act.Identity, bias=nth_t[:, 0:1], scale=1.0)
                                    nc.scalar.dma_start(od_ds[c][:, k - nw, q3:], ob[:, q3:])
                            else:
                                ob = ocp.tile([fl, tc, frd], bf16, tag=f"ob{c}", name=f"ob{c}_{k}")
                                nc.scalar.activation(ob[:], its[c][:], Act.Identity, bias=nth_t[:, 0:1], scale=1.0)
                                nc.scalar.dma_start(od_ds[c][:, k - nw], ob[:])
                else:
                    # Flush pool output chunk k-2 now: its compute finished
                    # long ago, so this never blocks the SP ring head.
                    if k - 2 >= nwp:
                        for c in range(2):
                            j = k - 2 - nwp
                            sl = slice(j * tc_p, (j + 1) * tc_p)
                            nc.sync.dma_start(op_d[:, c, j], pout[c][:, sl])
                    pts = ptiles.pop(k)
                    for t in range(tc_p):
                        gt = k * tc_p + t
                        for c in range(2):
                            dst = pscr[c][:, gt % 2] if gt < w else pout[c][:, gt - w]
                            if gt == 0:
                                # v=0: vp' == I, one copy op
                                nc.gpsimd.tensor_scalar(
                                    dst, pts[c][:, t], 0.0, None, Alu.add
                                )
                                continue
                            if gt - 1 < w:
                                st = pscr[c][:, (gt - 1) % 2]
                            else:
                                st = pout[c][:, gt - 1 - w]
                            # m = (vp < thr_t) * alpha
                            nc.gpsimd.tensor_scalar(
                                pm[c][:], st, th_ap, al_ap,
                                Alu.is_lt, Alu.mult,
                            )
                            # m = m * vp   (= alpha * v)
                            nc.gpsimd.tensor_tensor(pm[c][:], pm[c][:], st, Alu.mult)
                            # vp' = m + I_t
                            nc.gpsimd.tensor_tensor(dst, pm[c][:], pts[c][:, t], Alu.add)



    nc.compile()
    return nc


def _seg_table_v4(l_da: int, l_db: int, l_p: int):
    """Per quarter q: 6 contiguous segments [A0 A1 B0 B1 P0 P1] in [q*512,..)."""
    segs_d = [0, l_da, 2 * l_da, 2 * l_da + l_db]
    base_p = 2 * l_da + 2 * l_db
    segs_p = [base_p, base_p + l_p]
    return segs_d, segs_p


def _run_v4(I, alpha, thr_t, one_minus, w, _trace):
    global LAST_RESULTS, _CURRENT_NC
    from concourse.bass_utils import run_bass_kernel_spmd

    bl4, fl4 = B, 128
    tc, tc_p = 16, 16
    l_da, l_db, l_p = 112, 96, 48  # 2*(l_da + l_db + l_p) = 512 per quarter
    assert 2 * (l_da + l_db + l_p) == 512

    key = ("v4", bl4, fl4, w, l_da, l_db, l_p, tc, tc_p)
    if key not in _BUILD_CACHE:
        _BUILD_CACHE[key] = _build_v4(bl4, fl4, w, l_da, l_db, l_p, tc, tc_p)
    nc = _BUILD_CACHE[key]
    _CURRENT_NC = nc

    segs_d, segs_p = _seg_table_v4(l_da, l_db, l_p)
    l_ds = [l_da, l_db]
    nck_p = (w + l_p) // tc_p

    def pack(fsl, t0, length, tcx):
        """[fl, bl, w+length] zero-padded window ending at t0+length."""
        lo = max(0, t0 - w)
        pad = np.zeros((fl4, bl4, w + length), np.float32)
        pad[:, :, w - (t0 - lo):] = I[:, fsl, lo : t0 + length].transpose(1, 0, 2)
        nck = (w + length) // tcx
        return pad.reshape(fl4, bl4, nck, tcx).transpose(0, 2, 3, 1)  # [fl,nck,tcx,bl]

    in_maps = []
    for c in range(N_CORES):
        fg = c % 2
        fsl = slice(fg * fl4, (fg + 1) * fl4)
        q = c // 2
        base = q * 512
        im = {}
        for ch, nm in ((0, "i_dvea"), (1, "i_dveb")):
            nck = (w + l_ds[ch]) // tc
            i_dve = np.zeros((fl4, nck, tc, 2, bl4), np.float32)
            for u in range(2):
                t0 = base + segs_d[2 * ch + u]
                i_dve[:, :, :, u, :] = pack(fsl, t0, l_ds[ch], tc)
            im[nm] = i_dve.reshape(fl4, nck, tc, 2 * bl4)
        i_pool = np.zeros((fl4, 2, nck_p, tc_p, bl4), np.float32)
        for pc in range(2):
            t0 = base + segs_p[pc]
            i_pool[:, pc] = pack(fsl, t0, l_p, tc_p)
        im["i_pool"] = i_pool
        im["scal2"] = np.ascontiguousarray(
            np.stack([alpha[fsl], thr_t[fsl]], axis=1).astype(np.float32)
        )
        im["nthr_t"] = np.ascontiguousarray(-thr_t[fsl].reshape(fl4, 1))
        in_maps.append(im)

    res = run_bass_kernel_spmd(nc, in_maps, core_ids=list(range(N_CORES)), trace=_trace)
    LAST_RESULTS = res

    # DVE lane emits y = vt_pre - thr_t in bf16 (sign bit = exact spike
    # decision, incl. signed zero); pool lane emits vt_pre in f32.
    thr_col = thr_t.reshape(1, F, 1)
    vp = np.empty((B, F, L), np.float32)
    s = np.empty((B, F, L), np.float32)
    nsp = l_p // tc_p
    for c in range(N_CORES):
        fg = c % 2
        fsl = slice(fg * fl4, (fg + 1) * fl4)
        q = c // 2
        base = q * 512
        for ch, nm in ((0, "ya_out"), (1, "yb_out")):
            nsd = l_ds[ch] // tc
            rd = np.asarray(res.results[c][nm], np.float32).reshape(fl4, nsd, tc, 2, bl4)
            for u in range(2):
                t0 = base + segs_d[2 * ch + u]
                sl = slice(t0, t0 + l_ds[ch])
                a = rd[:, :, :, u, :].transpose(3, 0, 1, 2).reshape(bl4, fl4, l_ds[ch])
                s[:, fsl, sl] = ~np.signbit(a)
                vp[:, fsl, sl] = a + thr_t[fsl].reshape(1, fl4, 1)
        rp = np.asarray(res.results[c]["vpp_out"], np.float32).reshape(fl4, 2, nsp, tc_p, bl4)
        for pc in range(2):
            t0 = base + segs_p[pc]
            sl = slice(t0, t0 + l_p)
            a = rp[:, pc].transpose(3, 0, 1, 2).reshape(bl4, fl4, l_p)
            vp[:, fsl, sl] = a
            s[:, fsl, sl] = a >= thr_t[fsl].reshape(1, fl4, 1)

    om = one_minus.reshape(1, F, 1)
    v = (vp * om).astype(np.float32)
    z = ((v - np.float32(THR)) * np.float32(BETA)).astype(np.float32)
    return v, z, s


def _pick_warmup_v3(alpha: np.ndarray) -> int:
    """Warmup for the rescaled chain: decay the v=0 state error (~O(1) in
    the vt domain) below ~1e-7 absolute so spike decisions match a
    converged trajectory. Multiple of 16."""
    amax = float(alpha.max())
    amax = min(max(amax, 1e-6), 0.999999)
    wraw = np.log(5e-3) / np.log(amax)  # ~5.3 / -ln(amax)
    w = int(np.ceil(max(wraw, 1.0) / 16.0)) * 16
    return max(w, 16)


USE_V4 = True
USE_V3 = False  # _build_v3 no longer maintained; v2 is the fallback


def kernel(I: np.ndarray, raw_tau: np.ndarray, _trace: bool = False):
    I = np.asarray(I, dtype=np.float32)
    raw_tau = np.asarray(raw_tau, dtype=np.float32)
    assert I.shape == (B, F, L), I.shape

    alpha, one_minus = _alpha_host(raw_tau)
    w3 = _pick_warmup_v3(alpha)
    if USE_V4 and w3 <= 96:
        thr_t = (np.float32(THR) / one_minus).astype(np.float32)
        return _run_v4(I, alpha, thr_t, one_minus, w3, _trace)
    if USE_V3 and w3 <= 256:
        thr_t = (np.float32(THR) / one_minus).astype(np.float32)
        return _run_v3(I, alpha, thr_t, one_minus, w3, _trace)
    w = _pick_warmup(alpha)
    if USE_V2 and w <= 512:
        return _run_v2(I, alpha, one_minus, w, _trace)
    return _run_v1(I, alpha, one_minus, _trace)

